# revision 1
# baseline (speedup 1.0000x reference)
"""Causal attention (B=4, T=2048, D=1024) on 8 TRN2 NeuronCores.

Sharding: core c = (batch b = c//2, half h = c%2). Each core computes
attention for 1024 query rows of one batch: 4 slots of 256 rows, with
balanced causal work via block assignment h=0 -> blocks [7,4,3,0],
h=1 -> [6,5,2,1] (blocks of 256 rows). Slot s processes a k-tile
prefix of length CAPS[s] = [16,12,8,4] (k-tiles of 128 keys), which
covers both cores' needs exactly; the causal mask (generated on-device
from qpos/kpos) zeroes any over-computed region. K/V are computed for
the full batch on both cores of a batch (no cross-device comm).

Math per core (all matmul inputs bf16, fp32 PSUM accumulation):
  KT[e,t] = sum_d Wk[d,e] * xT[d,t]        (lhsT=Wk tile, rhs=xT)
  QT[e,q] = sum_d Wq[d,e] * xqT[d,q]
  V[t,e]  = sum_d xT[d,t] * Wv[d,e]        (lhsT=xT tile, rhs=Wv)
  ST[k,q] = sum_e KT[e,k] * QT[e,q]        (lhsT=KT tile, rhs=QT)
  PT[k,q] = exp(ST/sqrt(D)) * (qpos[q] >= kpos[k])   (no max-sub: logits ~N(0,1))
  O[q,e]  = sum_k PT[k,q] * V[k,e];  sum[q] = sum_k PT[k,q] (ones-column matmul)
  out[q,e] = O[q,e] / sum[q]
"""

import numpy as np
import ml_dtypes

import concourse.bacc as bacc
import concourse.bass as bass
import concourse.mybir as mybir
import concourse.tile as tile
from concourse.bass_utils import run_bass_kernel_spmd

BF16 = mybir.dt.bfloat16
F32 = mybir.dt.float32

B, T, D = 4, 2048, 1024
P = 128          # partitions
DT = D // P      # 8 d-tiles
KT_N = T // P    # 16 k-tiles
SLOT_Q = 256
NSLOT = 4
CAPS = [16, 12, 8, 4]           # k-tile prefix length per slot
OFF = [0, 16, 28, 36]           # unit offset per slot (cumsum of CAPS)
NUNIT = sum(CAPS)               # 40
ASSIGN = {0: [7, 4, 3, 0], 1: [6, 5, 2, 1]}   # q-block (of 256) per slot
SCALE = 1.0 / np.sqrt(np.float32(D))

_NC_CACHE = None


def _active(kt):
    """number of slots whose cap exceeds kt (slots are cap-descending)"""
    return sum(1 for c in CAPS if c > kt)


def build_nc(repeat=1, hw_loop=True):
    """repeat>1 replays the whole compute pipeline (stages B-F) that many
    times, reusing the loaded inputs — used only for differential wall-clock
    timing of the on-device execution (identical I/O footprint to repeat=1)."""
    nc = bacc.Bacc("TRN2", target_bir_lowering=False, debug=False,
                   enable_asserts=False, enable_partition_id=False)

    xkvT = nc.dram_tensor("xkvT", [D, T], BF16, kind="ExternalInput").ap()
    xqT = nc.dram_tensor("xqT", [D, NSLOT * SLOT_Q], BF16, kind="ExternalInput").ap()
    Wq = nc.dram_tensor("Wq", [D, D], BF16, kind="ExternalInput").ap()
    Wk = nc.dram_tensor("Wk", [D, D], BF16, kind="ExternalInput").ap()
    Wv = nc.dram_tensor("Wv", [D, D], BF16, kind="ExternalInput").ap()
    qpos_d = nc.dram_tensor("qpos", [NSLOT * SLOT_Q], F32, kind="ExternalInput").ap()
    kpos_d = nc.dram_tensor("kpos", [P, KT_N], F32, kind="ExternalInput").ap()
    out_d = nc.dram_tensor("out", [NSLOT * SLOT_Q, D], F32, kind="ExternalOutput").ap()

    NQ = NSLOT * SLOT_Q  # 1024 query rows per core
    # PT units reuse the W slots (tag "w") once the weights are dead; with
    # repeat>1 the weights stay live across reps, so PT needs its own tag.
    pt_tag = "w" if repeat == 1 else "pt"

    with tile.TileContext(nc) as tc:
        with tc.tile_pool(name="sb", bufs=1) as sb, \
             tc.tile_pool(name="ps", bufs=1, space="PSUM") as ps:

            # ---- stage A: load inputs ----
            wq_s = sb.tile([P, DT, D], BF16, tag="w", bufs=3)
            wk_s = sb.tile([P, DT, D], BF16, tag="w", bufs=3)
            wv_s = sb.tile([P, DT, D], BF16, tag="w", bufs=3)
            xkvT_s = sb.tile([P, DT, T], BF16, tag="xkvT", bufs=1)
            xqT_s = sb.tile([P, DT, NQ], BF16, tag="xqT", bufs=1)
            nc.sync.dma_start(out=wk_s, in_=Wk.rearrange("(dt p) e -> p dt e", p=P))
            _xr = xkvT.rearrange("(dt p) t -> p dt t", p=P)
            for _c in range(4):
                nc.sync.dma_start(out=xkvT_s[:, :, _c * 512:(_c + 1) * 512],
                                  in_=_xr[:, :, _c * 512:(_c + 1) * 512])
            nc.sync.dma_start(out=wq_s, in_=Wq.rearrange("(dt p) e -> p dt e", p=P))
            nc.sync.dma_start(out=xqT_s, in_=xqT.rearrange("(dt p) q -> p dt q", p=P))
            nc.sync.dma_start(out=wv_s, in_=Wv.rearrange("(dt p) e -> p dt e", p=P))

            qpos_s = sb.tile([P, NQ], F32, tag="qpos", bufs=1)
            qpos_bcast = bass.AP(tensor=qpos_d.tensor, offset=qpos_d.offset,
                                 ap=[[0, P]] + list(qpos_d.ap))
            nc.gpsimd.dma_start(out=qpos_s, in_=qpos_bcast)
            kpos_s = sb.tile([P, KT_N], F32, tag="kpos", bufs=1)
            nc.sync.dma_start(out=kpos_s, in_=kpos_d)
            ones_s = sb.tile([P, 1], BF16, tag="ones", bufs=1)
            nc.vector.memset(ones_s, 1.0)

            import contextlib
            n_emit = 1 if hw_loop else repeat
            _loop = (tc.For_i(0, repeat, 1) if (hw_loop and repeat > 1)
                     else contextlib.nullcontext())
            with _loop:
              for rep in range(n_emit):
                r = f"_{rep}" if n_emit > 1 else ""
                kt_s = sb.tile([P, DT, T], BF16, tag="kt", bufs=1, name=f"kt_s{r}")
                qt_s = sb.tile([P, DT, NQ], BF16, tag="qt", bufs=1, name=f"qt_s{r}")
                v_s = sb.tile([P, KT_N, D], BF16, tag="v", bufs=1, name=f"v_s{r}")

                # ---- stage B: KT[e,t] ----
                # 4 consecutive MMs share one lhsT (Wk tile) across two
                # paired PSUM slots -> one weight load per (et, dt)
                for et in range(DT):
                    pk0 = ps.tile([P, 1024], F32, tag="big", bufs=3,
                                  name=f"pk0{r}_{et}")
                    pk1 = ps.tile([P, 1024], F32, tag="big", bufs=3,
                                  name=f"pk1{r}_{et}")
                    pks = (pk0, pk1)
                    for dt in range(DT):
                        for hh in range(4):
                            t0 = hh * 512
                            nc.tensor.matmul(
                                pks[hh // 2][:, (hh % 2) * 512:(hh % 2 + 1) * 512],
                                wk_s[:, dt, et * P:(et + 1) * P],
                                xkvT_s[:, dt, t0:t0 + 512],
                                start=(dt == 0), stop=(dt == DT - 1))
                    nc.any.tensor_copy(out=kt_s[:, et, 0:1024], in_=pk0)
                    nc.any.tensor_copy(out=kt_s[:, et, 1024:2048], in_=pk1)

                # ---- stage C: QT[e,q] ----
                for et in range(DT):
                    pq = ps.tile([P, 1024], F32, tag="big", bufs=3,
                                 name=f"pq{r}_{et}")
                    for dt in range(DT):
                        for hh in range(NQ // 512):
                            nc.tensor.matmul(
                                pq[:, hh * 512:(hh + 1) * 512],
                                wq_s[:, dt, et * P:(et + 1) * P],
                                xqT_s[:, dt, hh * 512:(hh + 1) * 512],
                                start=(dt == 0), stop=(dt == DT - 1))
                    nc.any.tensor_copy(out=qt_s[:, et, :], in_=pq)

                # ---- stage D: V[t,e] ----
                for tt in range(KT_N):
                    pv = ps.tile([P, 1024], F32, tag="big", bufs=3,
                                 name=f"pv{r}_{tt}")
                    for dt in range(DT):
                        for hh in range(D // 512):
                            nc.tensor.matmul(
                                pv[:, hh * 512:(hh + 1) * 512],
                                xkvT_s[:, dt, tt * P:(tt + 1) * P],
                                wv_s[:, dt, hh * 512:(hh + 1) * 512],
                                start=(dt == 0), stop=(dt == DT - 1))
                    nc.any.tensor_copy(out=v_s[:, tt, :], in_=pv)

                # PT units: two halves sized to fit the reused "w" slots
                pt_bufs = 3 if repeat == 1 else 2
                pt_a = sb.tile([P, 20, SLOT_Q], BF16, tag=pt_tag, bufs=pt_bufs,
                               name=f"pt_a{r}")
                pt_b = sb.tile([P, 20, SLOT_Q], BF16, tag=pt_tag, bufs=pt_bufs,
                               name=f"pt_b{r}")

                def pt_unit(u, pt_a=pt_a, pt_b=pt_b):
                    return pt_a[:, u, :] if u < 20 else pt_b[:, u - 20, :]

                # ---- stage E: ST = KT.T @ QT per k-tile; PT = exp(ST*scale)*mask ----
                for kt in range(KT_N):
                    w = SLOT_Q * _active(kt)
                    st = ps.tile([P, 1024], F32, tag="big", bufs=3, name=f"st{r}_{kt}")
                    for dt in range(DT):
                        for p0 in range(0, w, 512):
                            pw = min(512, w - p0)
                            nc.tensor.matmul(
                                st[:, p0:p0 + pw],
                                kt_s[:, dt, kt * P:(kt + 1) * P],
                                qt_s[:, dt, p0:p0 + pw],
                                start=(dt == 0), stop=(dt == DT - 1))
                    for s in range(_active(kt)):
                        u = OFF[s] + kt
                        nc.scalar.activation(
                            out=pt_unit(u), in_=st[:, s * SLOT_Q:(s + 1) * SLOT_Q],
                            func=mybir.ActivationFunctionType.Exp, scale=float(SCALE))
                        # mask only where not provably all-keep for both cores
                        min_block = min(ASSIGN[0][s], ASSIGN[1][s])
                        if (kt + 1) * P > min_block * SLOT_Q:
                            m = sb.tile([P, SLOT_Q], BF16, tag="mask", bufs=4,
                                        name=f"m{r}_{kt}_{s}")
                            nc.vector.tensor_scalar(
                                out=m, in0=qpos_s[:, s * SLOT_Q:(s + 1) * SLOT_Q],
                                scalar1=kpos_s[:, kt:kt + 1], scalar2=None,
                                op0=mybir.AluOpType.is_ge)
                            nc.vector.tensor_mul(out=pt_unit(u), in0=pt_unit(u), in1=m)

                # ---- stage F: O = PT.T @ [V | 1]; normalize ----
                for s in range(NSLOT):
                    for qs in range(SLOT_Q // P):
                        po = ps.tile([P, 1024], F32, tag="big", bufs=3,
                                     name=f"po{r}_{s}_{qs}")
                        psum = ps.tile([P, 1], F32, tag="sum", bufs=2,
                                       name=f"psum{r}_{s}_{qs}")
                        for i, kt in enumerate(range(CAPS[s])):
                            lhsT = pt_unit(OFF[s] + kt)[:, qs * P:(qs + 1) * P]
                            fl = dict(start=(i == 0), stop=(i == CAPS[s] - 1))
                            nc.tensor.matmul(po[:, 0:512], lhsT, v_s[:, kt, 0:512], **fl)
                            nc.tensor.matmul(po[:, 512:1024], lhsT,
                                             v_s[:, kt, 512:1024], **fl)
                            nc.tensor.matmul(psum, lhsT, ones_s, **fl)
                        recip = sb.tile([P, 1], F32, tag="recip", bufs=4,
                                        name=f"rc{r}_{s}_{qs}")
                        nc.vector.reciprocal(out=recip, in_=psum)
                        o_sb = sb.tile([P, 1024], F32, tag="osb",
                                       bufs=(3 if repeat == 1 else 1),
                                       name=f"o{r}_{s}_{qs}")
                        nc.vector.tensor_scalar_mul(out=o_sb, in0=po[:, 0:1024],
                                                    scalar1=recip)
                        r0 = s * SLOT_Q + qs * P
                        nc.sync.dma_start(out=out_d[r0:r0 + P, :], in_=o_sb)

    nc.compile()
    return nc


def _host_prep(x, Wq, Wk, Wv):
    """Build per-core input maps. x: [B,T,D] fp32."""
    bf = ml_dtypes.bfloat16
    Wq_b, Wk_b, Wv_b = (np.ascontiguousarray(w.astype(bf)) for w in (Wq, Wk, Wv))
    kpos = (np.arange(T, dtype=np.float32).reshape(KT_N, P).T).copy()  # [P, KT_N]
    x_bf = x.astype(bf)                                    # [B, T, D], once
    xT_by_batch = [np.ascontiguousarray(x_bf[b].T) for b in range(B)]
    in_maps = []
    for c in range(8):
        b, h = divmod(c, 2)
        blocks = ASSIGN[h]
        xb = x_bf[b]                              # [T, D]
        xkvT = xT_by_batch[b]                     # [D, T] (shared by both cores)
        xq = np.concatenate([xb[g * SLOT_Q:(g + 1) * SLOT_Q] for g in blocks], axis=0)
        xqT = np.ascontiguousarray(xq.T)          # [D, 1024]
        qpos = np.concatenate([
            np.arange(g * SLOT_Q, (g + 1) * SLOT_Q, dtype=np.float32) for g in blocks])
        in_maps.append({
            "xkvT": xkvT, "xqT": xqT,
            "Wq": Wq_b, "Wk": Wk_b, "Wv": Wv_b,
            "qpos": qpos, "kpos": kpos,
        })
    return in_maps


def _reassemble(results, dtype=np.float32):
    out = np.empty((B, T, D), dtype=dtype)
    for c in range(8):
        b, h = divmod(c, 2)
        o = results[c]["out"]                     # [1024, D]
        for s, g in enumerate(ASSIGN[h]):
            out[b, g * SLOT_Q:(g + 1) * SLOT_Q] = o[s * SLOT_Q:(s + 1) * SLOT_Q]
    return out


def kernel(**inputs):
    global _NC_CACHE
    x = np.asarray(inputs["x"], dtype=np.float32)
    Wq = np.asarray(inputs["Wq"], dtype=np.float32)
    Wk = np.asarray(inputs["Wk"], dtype=np.float32)
    Wv = np.asarray(inputs["Wv"], dtype=np.float32)
    if _NC_CACHE is None:
        _NC_CACHE = build_nc()
    nc = _NC_CACHE
    in_maps = _host_prep(x, Wq, Wk, Wv)
    res = run_bass_kernel_spmd(nc, in_maps, core_ids=list(range(8)))
    return _reassemble(res.results)


if __name__ == "__main__":
    rng = np.random.default_rng(0)
    x = rng.standard_normal((B, T, D), dtype=np.float32)
    Wq = rng.standard_normal((D, D), dtype=np.float32) / np.sqrt(D)
    Wk = rng.standard_normal((D, D), dtype=np.float32) / np.sqrt(D)
    Wv = rng.standard_normal((D, D), dtype=np.float32) / np.sqrt(D)
    out = kernel(x=x, Wq=Wq, Wk=Wk, Wv=Wv)
    print("out", out.shape, out.dtype, np.abs(out).max())



# revision 28
# speedup vs baseline: 2.2670x; 2.2670x over previous
"""Causal attention (B=4, T=2048, D=1024) on 8 TRN2 NeuronCores.

Algorithm (folded projections; exact math, associativity only):
  S  = (x_q Wq)(x Wk)^T / sqrt(D) = x_q W' x^T,   W' = Wq Wk^T / sqrt(D)
  P  = exp(S) * causal_mask
  O  = (P (x Wv)) / rowsum(P) = ((P x) Wv) / rowsum(P)
K and V are never materialized: the full-T K/V projections (4.3 GF each
per core) are replaced by queries-only projections (2.1 GF each), taking
per-core matmul work from ~16.1 GF to ~9.7 GF.

Sharding: core c = (batch b = c//2, half h = c%2). Queries are split in
16 blocks of 128 rows; slot s of a core holds block 15-2s (h=0) or
14-2s (h=1), so both cores of a batch need the same causal k-tile
prefix CAPS[s] = 16-2s per slot (72 k-tile units total, the SPMD-common
cover). The causal mask (qpos >= kpos, on-device) zeroes the two
boundary tiles per slot; interior tiles are provably all-keep.

Per-core stages (all matmul inputs bf16, fp32 PSUM accumulation):
  Q''T[d',q] = sum_d W'[d,d'] xqT[d,q]          (lhsT=W' tile, rhs=xqT)
  ST[k,q]    = sum_d xT[d,k-tile] Q''T[d,q]     (lhsT=xT tile, rhs=Q''T)
  PT[k,q]    = exp(ST) * (qpos[q] >= kpos[k])
  PXT[d,q]   = sum_k x[k,d-chunk] PT[k,q]       (lhsT=x tile, rhs=PT)
  rowsum[q]  = sum_k PT[k,q]                    (lhsT=PT unit, rhs=ones)
  O[q,e]     = sum_d PXT[d,q] Wv[d,e]           (lhsT=PXT tile, rhs=Wv)
  out[q,e]   = O[q,e] / rowsum[q]
"""

import numpy as np
import ml_dtypes

import concourse.bacc as bacc
import concourse.bass as bass
import concourse.mybir as mybir
import concourse.tile as tile
from concourse.bass_utils import run_bass_kernel_spmd

BF16 = mybir.dt.bfloat16
F32 = mybir.dt.float32

B, T, D = 4, 2048, 1024
P = 128          # partitions
DT = D // P      # 8 d-tiles
KT_N = T // P    # 16 k-tiles
QB = 64          # query block rows
NSLOT = 16       # slots per core (16 x 64 = 1024 queries)
NQ = NSLOT * QB
CAPS = [16 - i for i in range(NSLOT)]              # k-tile prefix per slot
ASSIGN = {0: [31 - 2 * i for i in range(NSLOT)],   # global q-block per slot
          1: [30 - 2 * i for i in range(NSLOT)]}
SCALE = 1.0 / np.sqrt(np.float32(D))


def _active(kt):
    """number of slots whose cap exceeds kt (slots are cap-descending)"""
    return sum(1 for c in CAPS if c > kt)


# kt-major PT unit layout: unit (kt, s) lives at column UBASE[kt] + s
UBASE = np.cumsum([0] + [_active(k) for k in range(KT_N)]).tolist()
NUNIT = UBASE[KT_N]  # 72

_NC_CACHE = None


def build_nc(repeat=1, hw_loop=True):
    """repeat>1 replays the compute pipeline that many times reusing the
    loaded inputs — used only for differential wall-clock timing."""
    nc = bacc.Bacc("TRN2", target_bir_lowering=False, debug=False,
                   enable_asserts=False, enable_partition_id=False)

    xT_d = nc.dram_tensor("xT", [D, T], BF16, kind="ExternalInput").ap()
    x_d = nc.dram_tensor("x", [T, D], BF16, kind="ExternalInput").ap()
    xqT_d = nc.dram_tensor("xqT", [D, NQ], BF16, kind="ExternalInput").ap()
    Wp = nc.dram_tensor("Wp", [D, D], BF16, kind="ExternalInput").ap()
    Wv = nc.dram_tensor("Wv", [D, D], BF16, kind="ExternalInput").ap()
    qpos_d = nc.dram_tensor("qpos", [NQ], F32, kind="ExternalInput").ap()
    kpos_d = nc.dram_tensor("kpos", [P, KT_N], F32, kind="ExternalInput").ap()
    out_d = nc.dram_tensor("out", [NQ, D], F32, kind="ExternalOutput").ap()

    with tile.TileContext(nc) as tc:
        with tc.tile_pool(name="sb", bufs=1) as sb, \
             tc.tile_pool(name="ps", bufs=1, space="PSUM") as ps:

            # ---- stage A: load inputs (per-dt tiles so compute streams) ----
            _wpr = Wp.rearrange("(dt p) e -> p dt e", p=P)
            _xqr = xqT_d.rearrange("(dt p) q -> p dt q", p=P)
            _xtr = xT_d.rearrange("(dt p) t -> p dt t", p=P)
            _xr = x_d.rearrange("(kt p) d -> p kt d", p=P)
            # tiny starter tiles on two sequencers so the first Q'' matmuls
            # are not gated on the full first-dt transfers
            wp_start = sb.tile([P, 4 * P], BF16, tag="wps", bufs=1, name="wps")
            nc.sync.dma_start(out=wp_start, in_=_wpr[:, 0, 0:4 * P])
            xq_start = sb.tile([P, 512], BF16, tag="xqs", bufs=1, name="xqs")
            nc.gpsimd.dma_start(out=xq_start, in_=_xqr[:, 0, 0:512])
            wp_t, xqT_t = [], []
            for dt in range(DT):
                w = sb.tile([P, D], BF16, tag=f"wp{dt}", bufs=1, name=f"wp{dt}")
                nc.sync.dma_start(out=w, in_=_wpr[:, dt, :])
                wp_t.append(w)
                xq = sb.tile([P, NQ], BF16, tag=f"xq{dt}", bufs=1, name=f"xq{dt}")
                nc.sync.dma_start(out=xq, in_=_xqr[:, dt, :])
                xqT_t.append(xq)
            kpos_s = sb.tile([P, KT_N], F32, tag="kpos", bufs=1)
            nc.sync.dma_start(out=kpos_s, in_=kpos_d)
            # qpos broadcast is 512KB on the shared DMA bus and is not needed
            # until the first mask (~40us in): issue on the SP queue so it
            # stays ordered behind the critical head DMAs (the Pool sequencer
            # would start it immediately)
            qpos_s = sb.tile([P, NQ], F32, tag="qpos", bufs=1)
            qpos_bcast = bass.AP(tensor=qpos_d.tensor, offset=qpos_d.offset,
                                 ap=[[0, P]] + list(qpos_d.ap))
            nc.sync.dma_start(out=qpos_s, in_=qpos_bcast)
            xT_t = []
            for dt in range(DT):
                t = sb.tile([P, T], BF16, tag=f"xt{dt}", bufs=1, name=f"xt{dt}")
                nc.sync.dma_start(out=t, in_=_xtr[:, dt, :])
                xT_t.append(t)
            x_t = []
            for g in range(4):
                t = sb.tile([P, 4, D], BF16, tag=f"x{g}", bufs=1, name=f"x{g}")
                nc.sync.dma_start(out=t, in_=_xr[:, 4 * g:4 * (g + 1), :])
                x_t.append(t)
            wv_s = sb.tile([P, DT, D], BF16, tag="wv", bufs=1)
            nc.sync.dma_start(out=wv_s, in_=Wv.rearrange("(dt p) e -> p dt e", p=P))
            ones_s = sb.tile([P, 1], BF16, tag="ones", bufs=1)
            nc.vector.memset(ones_s, 1.0)
            # dummy activation: pulls the 1.3us act-table load into the
            # DMA-wait window instead of delaying the first real Act op
            warm_s = sb.tile([P, 1], F32, tag="warm", bufs=1)
            nc.scalar.copy(out=warm_s, in_=ones_s)

            import contextlib
            n_emit = 1 if hw_loop else repeat
            _loop = (tc.For_i(0, repeat, 1) if (hw_loop and repeat > 1)
                     else contextlib.nullcontext())
            with _loop:
              for rep in range(n_emit):
                r = f"_{rep}" if n_emit > 1 else ""
                qt_s = sb.tile([P, DT, NQ], BF16, tag="qt", bufs=1, name=f"qt{r}")
                pt_s = sb.tile([P, NUNIT * QB], BF16, tag="pt", bufs=1,
                               name=f"pt{r}")

                # ---- stage B: Q''T[d',q] ----
                # first three et-chunks run dt-major so the PE streams behind
                # the wp/xqT DMA arrivals; the rest run after the DMA is done
                pq3 = [ps.tile([P, 1024], F32, tag="big", bufs=4,
                               name=f"pq{r}_{et}") for et in range(4)]
                for dt in range(DT):
                    for et in range(4):
                        lhsT = (wp_start[:, et * P:(et + 1) * P]
                                if (dt == 0 and rep == 0)
                                else wp_t[dt][:, et * P:(et + 1) * P])
                        for qh in range(2):
                            rhs = (xq_start if (dt == 0 and qh == 0 and rep == 0)
                                   else xqT_t[dt][:, qh * 512:(qh + 1) * 512])
                            nc.tensor.matmul(
                                pq3[et][:, qh * 512:(qh + 1) * 512],
                                lhsT, rhs,
                                start=(dt == 0), stop=(dt == DT - 1))
                def qt_copy(et, pq):
                    # split across DVE and Act so the psum buf frees fast
                    nc.vector.tensor_copy(out=qt_s[:, et, 0:512],
                                          in_=pq[:, 0:512])
                    nc.scalar.copy(out=qt_s[:, et, 512:1024],
                                   in_=pq[:, 512:1024])

                for et in range(4):
                    qt_copy(et, pq3[et])
                for et in range(4, DT):
                    pq = ps.tile([P, 1024], F32, tag="big", bufs=4,
                                 name=f"pq{r}_{et}")
                    for dt in range(DT):
                        for qh in range(2):
                            nc.tensor.matmul(
                                pq[:, qh * 512:(qh + 1) * 512],
                                wp_t[dt][:, et * P:(et + 1) * P],
                                xqT_t[dt][:, qh * 512:(qh + 1) * 512],
                                start=(dt == 0), stop=(dt == DT - 1))
                    qt_copy(et, pq)

                # ---- stage C: ST = xT.T @ Q''T per k-tile; PT = exp(ST)*mask
                for kt in range(KT_N):
                    a = _active(kt)
                    groups = [(0, min(8, a))] + ([(8, a)] if a > 8 else [])
                    for (lo, hi) in groups:
                        w = (hi - lo) * QB
                        st = ps.tile([P, 512], F32, tag="big", bufs=4,
                                     name=f"st{r}_{kt}_{lo}")
                        for dt in range(DT):
                            nc.tensor.matmul(
                                st[:, 0:w],
                                xT_t[dt][:, kt * P:(kt + 1) * P],
                                qt_s[:, dt, lo * QB:hi * QB],
                                start=(dt == 0), stop=(dt == DT - 1))
                        u0 = UBASE[kt] + lo
                        nc.scalar.activation(
                            out=pt_s[:, u0 * QB:u0 * QB + w], in_=st[:, 0:w],
                            func=mybir.ActivationFunctionType.Exp, scale=1.0)
                        # boundary tiles of the last two active slots need
                        # the causal mask; interior tiles are all-keep
                        for s in (a - 2, a - 1):
                            if not (lo <= s < hi):
                                continue
                            u = UBASE[kt] + s
                            m = sb.tile([P, QB], BF16, tag="mask", bufs=4,
                                        name=f"m{r}_{kt}_{s}")
                            nc.vector.tensor_scalar(
                                out=m, in0=qpos_s[:, s * QB:(s + 1) * QB],
                                scalar1=kpos_s[:, kt:kt + 1], scalar2=None,
                                op0=mybir.AluOpType.is_ge)
                            nc.vector.tensor_mul(
                                out=pt_s[:, u * QB:(u + 1) * QB],
                                in0=pt_s[:, u * QB:(u + 1) * QB], in1=m)

                # ---- stage D/E: PXT accum per slot; rowsum; O = PXT.T @ Wv
                def pt_unit(s, kt):
                    u = UBASE[kt] + s
                    return pt_s[:, u * QB:(u + 1) * QB]

                def emit_px(s, px_sb_pair, psum_pair):
                    """slot s of a pair: even slot -> q cols/rows [0:64] of
                    the pair tiles, odd slot -> [64:128]"""
                    cap = CAPS[s]
                    half = s % 2
                    px = ps.tile([P, DT, QB], F32, tag="big", bufs=4,
                                 name=f"px{r}_{s}")
                    # dc outer: a start_tensor_calc marks the whole 2KB psum
                    # zero-region pending, so dc slices sharing a bank must
                    # run their groups sequentially, not interleaved
                    for dc in range(DT):
                        for i, kt in enumerate(range(cap)):
                            nc.tensor.matmul(
                                px[:, dc, :],
                                x_t[kt // 4][:, kt % 4, dc * P:(dc + 1) * P],
                                pt_unit(s, kt),
                                start=(i == 0), stop=(i == cap - 1))
                    # rowsum lands in the pair's partition half (q rows)
                    tgt = psum_pair[half * QB:(half + 1) * QB, :]
                    for i, kt in enumerate(range(cap)):
                        nc.tensor.matmul(tgt, pt_unit(s, kt), ones_s,
                                         start=(i == 0), stop=(i == cap - 1))
                    # two half-copies on separate engines: halves the copy
                    # latency and lets O's dt=0..3 matmuls start after the
                    # first half lands
                    c0 = half * QB
                    nc.vector.tensor_copy(out=px_sb_pair[:, 0:4, c0:c0 + QB],
                                          in_=px[:, 0:4, :])
                    nc.scalar.copy(out=px_sb_pair[:, 4:8, c0:c0 + QB],
                                   in_=px[:, 4:8, :])

                def emit_o(p, px_sb, recip, tail=False):
                    o_sb = sb.tile([P, 1024], F32, tag="osb", bufs=2,
                                   name=f"o{r}_{p}")
                    r0 = p * 2 * QB
                    # per-chunk psum tiles: a chunk's group start would
                    # otherwise serialize (tile-level WAR) against the
                    # previous chunk's normalize read; tail=True uses quarter
                    # chunks so the final DMA overlaps remaining matmuls
                    chunks = ([(0, 384), (384, 640), (640, 896), (896, 1024)]
                              if tail else [(0, 512), (512, 1024)])
                    nch = len(chunks)
                    for h, (c0, c1) in enumerate(chunks):
                        po = ps.tile([P, c1 - c0], F32, tag="big", bufs=4,
                                     name=f"po{r}_{p}_{h}")
                        for dt in range(DT):
                            nc.tensor.matmul(
                                po, px_sb[:, dt, :], wv_s[:, dt, c0:c1],
                                start=(dt == 0), stop=(dt == DT - 1))
                        # the last chunk takes the faster DVE + SP-queue chain
                        if (nch - 1 - h) % 2 == 0:
                            nc.vector.tensor_scalar_mul(
                                out=o_sb[:, c0:c1], in0=po, scalar1=recip)
                            dma = nc.sync.dma_start
                        else:
                            nc.scalar.mul(out=o_sb[:, c0:c1], in_=po,
                                          mul=recip)
                            dma = nc.scalar.dma_start
                        dma(out=out_d[r0:r0 + 2 * QB, c0:c1],
                            in_=o_sb[:, c0:c1])

                # slots processed small-cap first; a pair's O-projection is
                # emitted one pair later (lag-1) so its px->sbuf copies
                # overlap the next pair's PX matmuls
                pend = None
                cur = None
                for s in range(NSLOT - 1, -1, -1):
                    if s % 2 == 1:
                        p = s // 2
                        px_sb_pair = sb.tile([P, DT, 2 * QB], BF16,
                                             tag="pxsb", bufs=3,
                                             name=f"pxsb{r}_{p}")
                        psum_pair = ps.tile([P, 1], F32, tag="big", bufs=4,
                                            name=f"psum{r}_{p}")
                        cur = (p, px_sb_pair, psum_pair)
                    emit_px(s, cur[1], cur[2])
                    if s % 2 == 0:
                        p, px_sb_pair, psum_pair = cur
                        recip = sb.tile([P, 1], F32, tag="recip", bufs=4,
                                        name=f"rc{r}_{p}")
                        nc.vector.reciprocal(out=recip, in_=psum_pair)
                        if pend is not None:
                            emit_o(*pend)
                        pend = (p, px_sb_pair, recip)
                emit_o(*pend, tail=True)

    nc.compile()
    return nc


def _host_prep(x, Wq, Wk, Wv):
    """Build per-core input maps. x: [B,T,D] fp32."""
    bf = ml_dtypes.bfloat16
    Wp = np.ascontiguousarray(((Wq @ Wk.T) * SCALE).astype(bf))
    Wv_b = np.ascontiguousarray(Wv.astype(bf))
    kpos = (np.arange(T, dtype=np.float32).reshape(KT_N, P).T).copy()
    x_bf = x.astype(bf)                                    # [B, T, D], once
    xT_by_batch = [np.ascontiguousarray(x_bf[b].T) for b in range(B)]
    in_maps = []
    for c in range(8):
        b, h = divmod(c, 2)
        blocks = ASSIGN[h]
        xb = x_bf[b]                              # [T, D]
        xq = np.concatenate([xb[j * QB:(j + 1) * QB] for j in blocks], axis=0)
        xqT = np.ascontiguousarray(xq.T)          # [D, 1024]
        qpos = np.concatenate([
            np.arange(j * QB, (j + 1) * QB, dtype=np.float32) for j in blocks])
        in_maps.append({
            "xT": xT_by_batch[b], "x": xb, "xqT": xqT,
            "Wp": Wp, "Wv": Wv_b,
            "qpos": qpos, "kpos": kpos,
        })
    return in_maps


def _reassemble(results, dtype=np.float32):
    out = np.empty((B, T, D), dtype=dtype)
    for c in range(8):
        b, h = divmod(c, 2)
        o = results[c]["out"]                     # [1024, D]
        for s, j in enumerate(ASSIGN[h]):
            out[b, j * QB:(j + 1) * QB] = o[s * QB:(s + 1) * QB]
    return out


def kernel(**inputs):
    global _NC_CACHE
    x = np.asarray(inputs["x"], dtype=np.float32)
    Wq = np.asarray(inputs["Wq"], dtype=np.float32)
    Wk = np.asarray(inputs["Wk"], dtype=np.float32)
    Wv = np.asarray(inputs["Wv"], dtype=np.float32)
    if _NC_CACHE is None:
        _NC_CACHE = build_nc()
    nc = _NC_CACHE
    in_maps = _host_prep(x, Wq, Wk, Wv)
    res = run_bass_kernel_spmd(nc, in_maps, core_ids=list(range(8)))
    return _reassemble(res.results)


if __name__ == "__main__":
    rng = np.random.default_rng(0)
    x = rng.standard_normal((B, T, D), dtype=np.float32)
    Wq = rng.standard_normal((D, D), dtype=np.float32) / np.sqrt(D)
    Wk = rng.standard_normal((D, D), dtype=np.float32) / np.sqrt(D)
    Wv = rng.standard_normal((D, D), dtype=np.float32) / np.sqrt(D)
    out = kernel(x=x, Wq=Wq, Wk=Wk, Wv=Wv)
    print("out", out.shape, out.dtype, np.abs(out).max())


# revision 38
# speedup vs baseline: 2.2855x; 1.0082x over previous
"""Causal attention (B=4, T=2048, D=1024) on 8 TRN2 NeuronCores.

Algorithm (folded projections; exact math, associativity only):
  S  = (x_q Wq)(x Wk)^T / sqrt(D) = x_q W' x^T,   W' = Wq Wk^T / sqrt(D)
  P  = exp(S) * causal_mask
  O  = (P (x Wv)) / rowsum(P) = ((P x) Wv) / rowsum(P)
K and V are never materialized: the full-T K/V projections (4.3 GF each
per core) are replaced by queries-only projections (2.1 GF each), taking
per-core matmul work from ~16.1 GF to ~9.7 GF.

Sharding: core c = (batch b = c//2, half h = c%2). Queries are split in
32 blocks of 64 rows; slot s (0..15) of a core holds block 31-2s (h=0)
or 30-2s (h=1), so both cores of a batch need the same causal k-tile
prefix CAPS[s] = 16-s per slot (136 [128k x 64q] units total — exactly
the causal minimum for this query interleaving). The causal mask
(qpos >= kpos, on-device) zeroes the two boundary tiles per slot;
interior tiles are provably all-keep. Slot pairs (2p, 2p+1) share a
128-row output projection / rowsum / store.

Per-core stages (all matmul inputs bf16, fp32 PSUM accumulation):
  Q''T[d',q] = sum_d W'[d,d'] xqT[d,q]          (lhsT=W' tile, rhs=xqT)
  ST[k,q]    = sum_d xT[d,k-tile] Q''T[d,q]     (lhsT=xT tile, rhs=Q''T)
  PT[k,q]    = exp(ST) * (qpos[q] >= kpos[k])
  PXT[d,q]   = sum_k x[k,d-chunk] PT[k,q]       (lhsT=x tile, rhs=PT)
  rowsum[q]  = sum_k PT[k,q]                    (lhsT=PT unit, rhs=ones)
  O[q,e]     = sum_d PXT[d,q] Wv[d,e]           (lhsT=PXT tile, rhs=Wv)
  out[q,e]   = O[q,e] / rowsum[q]
"""

import numpy as np
import ml_dtypes

import concourse.bacc as bacc
import concourse.bass as bass
import concourse.mybir as mybir
import concourse.tile as tile
from concourse.bass_utils import run_bass_kernel_spmd

BF16 = mybir.dt.bfloat16
F32 = mybir.dt.float32

B, T, D = 4, 2048, 1024
P = 128          # partitions
DT = D // P      # 8 d-tiles
KT_N = T // P    # 16 k-tiles
QB = 64          # query block rows
NSLOT = 16       # slots per core (16 x 64 = 1024 queries)
NQ = NSLOT * QB
CAPS = [16 - i for i in range(NSLOT)]              # k-tile prefix per slot
ASSIGN = {0: [31 - 2 * i for i in range(NSLOT)],   # global q-block per slot
          1: [30 - 2 * i for i in range(NSLOT)]}
SCALE = 1.0 / np.sqrt(np.float32(D))


def _active(kt):
    """number of slots whose cap exceeds kt (slots are cap-descending)"""
    return sum(1 for c in CAPS if c > kt)


# kt-major PT unit layout: unit (kt, s) lives at column UBASE[kt] + s
UBASE = np.cumsum([0] + [_active(k) for k in range(KT_N)]).tolist()
NUNIT = UBASE[KT_N]  # 136

_NC_CACHE = None


def build_nc(repeat=1, hw_loop=True):
    """repeat>1 replays the compute pipeline that many times reusing the
    loaded inputs — used only for differential wall-clock timing."""
    nc = bacc.Bacc("TRN2", target_bir_lowering=False, debug=False,
                   enable_asserts=False, enable_partition_id=False)

    xT_d = nc.dram_tensor("xT", [D, T], BF16, kind="ExternalInput").ap()
    x_d = nc.dram_tensor("x", [T, D], BF16, kind="ExternalInput").ap()
    xqT_d = nc.dram_tensor("xqT", [D, NQ], BF16, kind="ExternalInput").ap()
    Wp = nc.dram_tensor("Wp", [D, D], BF16, kind="ExternalInput").ap()
    Wv = nc.dram_tensor("Wv", [D, D], BF16, kind="ExternalInput").ap()
    qpos_d = nc.dram_tensor("qpos", [NQ], F32, kind="ExternalInput").ap()
    kpos_d = nc.dram_tensor("kpos", [P, KT_N], F32, kind="ExternalInput").ap()
    out_d = nc.dram_tensor("out", [NQ, D], F32, kind="ExternalOutput").ap()

    with tile.TileContext(nc) as tc:
        with tc.tile_pool(name="sb", bufs=1) as sb, \
             tc.tile_pool(name="ps", bufs=1, space="PSUM") as ps:

            # ---- stage A: load inputs (per-dt tiles so compute streams) ----
            _wpr = Wp.rearrange("(dt p) e -> p dt e", p=P)
            _xqr = xqT_d.rearrange("(dt p) q -> p dt q", p=P)
            _xtr = xT_d.rearrange("(dt p) t -> p dt t", p=P)
            _xr = x_d.rearrange("(kt p) d -> p kt d", p=P)
            # tiny starter tiles on two sequencers so the first Q'' matmuls
            # are not gated on the full first-dt transfers
            wp_start = sb.tile([P, 4 * P], BF16, tag="wps", bufs=1, name="wps")
            nc.sync.dma_start(out=wp_start, in_=_wpr[:, 0, 0:4 * P])
            xq_start = sb.tile([P, 512], BF16, tag="xqs", bufs=1, name="xqs")
            nc.gpsimd.dma_start(out=xq_start, in_=_xqr[:, 0, 0:512])
            wp_t, xqT_t = [], []
            for dt in range(DT):
                w = sb.tile([P, D], BF16, tag=f"wp{dt}", bufs=1, name=f"wp{dt}")
                xq = sb.tile([P, NQ], BF16, tag=f"xq{dt}", bufs=1, name=f"xq{dt}")
                if dt == 0:
                    # dt0 cols 0:512 of wp are covered by the starter (and
                    # never read again); xq0 qh0 likewise — only fetch the
                    # halves the stream actually reads, saving early bus time
                    nc.sync.dma_start(out=xq[:, 512:1024],
                                      in_=_xqr[:, 0, 512:1024])
                else:
                    nc.sync.dma_start(out=w, in_=_wpr[:, dt, :])
                    nc.sync.dma_start(out=xq, in_=_xqr[:, dt, :])
                wp_t.append(w)
                xqT_t.append(xq)
            # wp dt0 cols 512:1024 (read by the non-streamed ets at ~18us)
            nc.sync.dma_start(out=wp_t[0][:, 512:1024],
                              in_=_wpr[:, 0, 512:1024])
            kpos_s = sb.tile([P, KT_N], F32, tag="kpos", bufs=1)
            nc.sync.dma_start(out=kpos_s, in_=kpos_d)
            # qpos broadcast is 512KB on the shared DMA bus and is not needed
            # until the first mask (~40us in): issue on the SP queue so it
            # stays ordered behind the critical head DMAs (the Pool sequencer
            # would start it immediately)
            qpos_s = sb.tile([P, NQ], F32, tag="qpos", bufs=1)
            qpos_bcast = bass.AP(tensor=qpos_d.tensor, offset=qpos_d.offset,
                                 ap=[[0, P]] + list(qpos_d.ap))
            nc.sync.dma_start(out=qpos_s, in_=qpos_bcast)
            xT_t = []
            for dt in range(DT):
                t = sb.tile([P, T], BF16, tag=f"xt{dt}", bufs=1, name=f"xt{dt}")
                nc.sync.dma_start(out=t, in_=_xtr[:, dt, :])
                xT_t.append(t)
            x_t = []
            for g in range(4):
                t = sb.tile([P, 4, D], BF16, tag=f"x{g}", bufs=1, name=f"x{g}")
                nc.sync.dma_start(out=t, in_=_xr[:, 4 * g:4 * (g + 1), :])
                x_t.append(t)
            wv_s = sb.tile([P, DT, D], BF16, tag="wv", bufs=1)
            nc.sync.dma_start(out=wv_s, in_=Wv.rearrange("(dt p) e -> p dt e", p=P))
            ones_s = sb.tile([P, 1], BF16, tag="ones", bufs=1)
            nc.vector.memset(ones_s, 1.0)
            # dummy activation: pulls the 1.3us act-table load into the
            # DMA-wait window instead of delaying the first real Act op
            warm_s = sb.tile([P, 1], F32, tag="warm", bufs=1)
            nc.scalar.copy(out=warm_s, in_=ones_s)

            import contextlib
            n_emit = 1 if hw_loop else repeat
            _loop = (tc.For_i(0, repeat, 1) if (hw_loop and repeat > 1)
                     else contextlib.nullcontext())
            with _loop:
              for rep in range(n_emit):
                r = f"_{rep}" if n_emit > 1 else ""
                qt_s = sb.tile([P, DT, NQ], BF16, tag="qt", bufs=1, name=f"qt{r}")
                pt_s = sb.tile([P, NUNIT * QB], BF16, tag="pt", bufs=1,
                               name=f"pt{r}")

                # ---- stage B: Q''T[d',q] ----
                # first four et-chunks run dt-major so the PE streams behind
                # the wp/xqT DMA arrivals; the rest run after the DMA is done
                pq3 = [ps.tile([P, 1024], F32, tag="big", bufs=4,
                               name=f"pq{r}_{et}") for et in range(4)]
                for dt in range(DT):
                    for et in range(4):
                        lhsT = (wp_start[:, et * P:(et + 1) * P]
                                if (dt == 0 and rep == 0)
                                else wp_t[dt][:, et * P:(et + 1) * P])
                        for qh in range(2):
                            rhs = (xq_start if (dt == 0 and qh == 0 and rep == 0)
                                   else xqT_t[dt][:, qh * 512:(qh + 1) * 512])
                            nc.tensor.matmul(
                                pq3[et][:, qh * 512:(qh + 1) * 512],
                                lhsT, rhs,
                                start=(dt == 0), stop=(dt == DT - 1))
                def qt_copy(et, pq):
                    # split across DVE and Act so the psum buf frees fast
                    nc.vector.tensor_copy(out=qt_s[:, et, 0:512],
                                          in_=pq[:, 0:512])
                    nc.scalar.copy(out=qt_s[:, et, 512:1024],
                                   in_=pq[:, 512:1024])

                for et in range(4):
                    qt_copy(et, pq3[et])
                for et in range(4, DT):
                    pq = ps.tile([P, 1024], F32, tag="big", bufs=4,
                                 name=f"pq{r}_{et}")
                    for dt in range(DT):
                        for qh in range(2):
                            rhs = (xq_start if (dt == 0 and qh == 0)
                                   else xqT_t[dt][:, qh * 512:(qh + 1) * 512])
                            nc.tensor.matmul(
                                pq[:, qh * 512:(qh + 1) * 512],
                                wp_t[dt][:, et * P:(et + 1) * P],
                                rhs,
                                start=(dt == 0), stop=(dt == DT - 1))
                    qt_copy(et, pq)

                # ---- stage C: ST = xT.T @ Q''T per k-tile; PT = exp(ST)*mask
                # the two smallest k-tiles are emitted early so their exp
                # latency hides behind the remaining groups instead of
                # stalling the first PX allocations
                _kt_order = list(range(12)) + [14, 15, 12, 13]
                for kt in _kt_order:
                    a = _active(kt)
                    groups = [(0, min(8, a))] + ([(8, a)] if a > 8 else [])
                    for (lo, hi) in groups:
                        w = (hi - lo) * QB
                        st = ps.tile([P, 512], F32, tag="big", bufs=4,
                                     name=f"st{r}_{kt}_{lo}")
                        for dt in range(DT):
                            nc.tensor.matmul(
                                st[:, 0:w],
                                xT_t[dt][:, kt * P:(kt + 1) * P],
                                qt_s[:, dt, lo * QB:hi * QB],
                                start=(dt == 0), stop=(dt == DT - 1))
                        u0 = UBASE[kt] + lo
                        nc.scalar.activation(
                            out=pt_s[:, u0 * QB:u0 * QB + w], in_=st[:, 0:w],
                            func=mybir.ActivationFunctionType.Exp, scale=1.0)
                        # boundary tiles of the last two active slots need
                        # the causal mask; interior tiles are all-keep
                        for s in (a - 2, a - 1):
                            if not (lo <= s < hi):
                                continue
                            u = UBASE[kt] + s
                            m = sb.tile([P, QB], BF16, tag="mask", bufs=4,
                                        name=f"m{r}_{kt}_{s}")
                            nc.vector.tensor_scalar(
                                out=m, in0=qpos_s[:, s * QB:(s + 1) * QB],
                                scalar1=kpos_s[:, kt:kt + 1], scalar2=None,
                                op0=mybir.AluOpType.is_ge)
                            nc.vector.tensor_mul(
                                out=pt_s[:, u * QB:(u + 1) * QB],
                                in0=pt_s[:, u * QB:(u + 1) * QB], in1=m)

                # ---- stage D/E: PXT accum per slot; rowsum; O = PXT.T @ Wv
                def pt_unit(s, kt):
                    u = UBASE[kt] + s
                    return pt_s[:, u * QB:(u + 1) * QB]

                def emit_px(s, px_sb_pair, psum_pair):
                    """slot s of a pair: even slot -> q cols/rows [0:64] of
                    the pair tiles, odd slot -> [64:128]"""
                    cap = CAPS[s]
                    half = s % 2
                    px = ps.tile([P, DT, QB], F32, tag="big", bufs=4,
                                 name=f"px{r}_{s}")
                    # dc outer: a start_tensor_calc marks the whole 2KB psum
                    # zero-region pending, so dc slices sharing a bank must
                    # run their groups sequentially, not interleaved
                    for dc in range(DT):
                        for i, kt in enumerate(range(cap)):
                            nc.tensor.matmul(
                                px[:, dc, :],
                                x_t[kt // 4][:, kt % 4, dc * P:(dc + 1) * P],
                                pt_unit(s, kt),
                                start=(i == 0), stop=(i == cap - 1))
                    # rowsum lands in the pair's partition half (q rows)
                    tgt = psum_pair[half * QB:(half + 1) * QB, :]
                    for i, kt in enumerate(range(cap)):
                        nc.tensor.matmul(tgt, pt_unit(s, kt), ones_s,
                                         start=(i == 0), stop=(i == cap - 1))
                    # two half-copies on separate engines: halves the copy
                    # latency and lets O's dt=0..3 matmuls start after the
                    # first half lands
                    c0 = half * QB
                    nc.vector.tensor_copy(out=px_sb_pair[:, 0:4, c0:c0 + QB],
                                          in_=px[:, 0:4, :])
                    nc.scalar.copy(out=px_sb_pair[:, 4:8, c0:c0 + QB],
                                   in_=px[:, 4:8, :])

                def emit_o(p, px_sb, recip, tail=False):
                    o_sb = sb.tile([P, 1024], F32, tag="osb", bufs=2,
                                   name=f"o{r}_{p}")
                    r0 = p * 2 * QB
                    # per-chunk psum tiles: a chunk's group start would
                    # otherwise serialize (tile-level WAR) against the
                    # previous chunk's normalize read; tail=True uses quarter
                    # chunks so the final DMA overlaps remaining matmuls
                    chunks = ([(0, 384), (384, 640), (640, 896), (896, 1024)]
                              if tail else [(0, 512), (512, 1024)])
                    nch = len(chunks)
                    for h, (c0, c1) in enumerate(chunks):
                        po = ps.tile([P, c1 - c0], F32, tag="big", bufs=4,
                                     name=f"po{r}_{p}_{h}")
                        for dt in range(DT):
                            nc.tensor.matmul(
                                po, px_sb[:, dt, :], wv_s[:, dt, c0:c1],
                                start=(dt == 0), stop=(dt == DT - 1))
                        # the last chunk takes the faster DVE + SP-queue chain
                        if (nch - 1 - h) % 2 == 0:
                            nc.vector.tensor_scalar_mul(
                                out=o_sb[:, c0:c1], in0=po, scalar1=recip)
                            dma = nc.sync.dma_start
                        else:
                            nc.scalar.mul(out=o_sb[:, c0:c1], in_=po,
                                          mul=recip)
                            dma = nc.scalar.dma_start
                        dma(out=out_d[r0:r0 + 2 * QB, c0:c1],
                            in_=o_sb[:, c0:c1])

                # slots processed small-cap first; a pair's O-projection is
                # emitted one pair later (lag-1) so its px->sbuf copies
                # overlap the next pair's PX matmuls
                pend = None
                cur = None
                for s in range(NSLOT - 1, -1, -1):
                    if s % 2 == 1:
                        p = s // 2
                        px_sb_pair = sb.tile([P, DT, 2 * QB], BF16,
                                             tag="pxsb", bufs=3,
                                             name=f"pxsb{r}_{p}")
                        psum_pair = ps.tile([P, 1], F32, tag="big", bufs=4,
                                            name=f"psum{r}_{p}")
                        cur = (p, px_sb_pair, psum_pair)
                    emit_px(s, cur[1], cur[2])
                    if s % 2 == 0:
                        p, px_sb_pair, psum_pair = cur
                        recip = sb.tile([P, 1], F32, tag="recip", bufs=4,
                                        name=f"rc{r}_{p}")
                        nc.vector.reciprocal(out=recip, in_=psum_pair)
                        if pend is not None:
                            emit_o(*pend)
                        pend = (p, px_sb_pair, recip)
                emit_o(*pend, tail=True)

    nc.compile()
    return nc


def _host_prep(x, Wq, Wk, Wv):
    """Build per-core input maps. x: [B,T,D] fp32."""
    bf = ml_dtypes.bfloat16
    Wp = np.ascontiguousarray(((Wq @ Wk.T) * SCALE).astype(bf))
    Wv_b = np.ascontiguousarray(Wv.astype(bf))
    kpos = (np.arange(T, dtype=np.float32).reshape(KT_N, P).T).copy()
    x_bf = x.astype(bf)                                    # [B, T, D], once
    xT_by_batch = [np.ascontiguousarray(x_bf[b].T) for b in range(B)]
    in_maps = []
    for c in range(8):
        b, h = divmod(c, 2)
        blocks = ASSIGN[h]
        xb = x_bf[b]                              # [T, D]
        xq = np.concatenate([xb[j * QB:(j + 1) * QB] for j in blocks], axis=0)
        xqT = np.ascontiguousarray(xq.T)          # [D, 1024]
        qpos = np.concatenate([
            np.arange(j * QB, (j + 1) * QB, dtype=np.float32) for j in blocks])
        in_maps.append({
            "xT": xT_by_batch[b], "x": xb, "xqT": xqT,
            "Wp": Wp, "Wv": Wv_b,
            "qpos": qpos, "kpos": kpos,
        })
    return in_maps


def _reassemble(results, dtype=np.float32):
    out = np.empty((B, T, D), dtype=dtype)
    for c in range(8):
        b, h = divmod(c, 2)
        o = results[c]["out"]                     # [1024, D]
        for s, j in enumerate(ASSIGN[h]):
            out[b, j * QB:(j + 1) * QB] = o[s * QB:(s + 1) * QB]
    return out


def kernel(**inputs):
    global _NC_CACHE
    x = np.asarray(inputs["x"], dtype=np.float32)
    Wq = np.asarray(inputs["Wq"], dtype=np.float32)
    Wk = np.asarray(inputs["Wk"], dtype=np.float32)
    Wv = np.asarray(inputs["Wv"], dtype=np.float32)
    if _NC_CACHE is None:
        _NC_CACHE = build_nc()
    nc = _NC_CACHE
    in_maps = _host_prep(x, Wq, Wk, Wv)
    res = run_bass_kernel_spmd(nc, in_maps, core_ids=list(range(8)))
    return _reassemble(res.results)


if __name__ == "__main__":
    rng = np.random.default_rng(0)
    x = rng.standard_normal((B, T, D), dtype=np.float32)
    Wq = rng.standard_normal((D, D), dtype=np.float32) / np.sqrt(D)
    Wk = rng.standard_normal((D, D), dtype=np.float32) / np.sqrt(D)
    Wv = rng.standard_normal((D, D), dtype=np.float32) / np.sqrt(D)
    out = kernel(x=x, Wq=Wq, Wk=Wk, Wv=Wv)
    print("out", out.shape, out.dtype, np.abs(out).max())


# revision 54
# speedup vs baseline: 2.6301x; 1.1507x over previous
"""Causal attention (B=4, T=2048, D=1024) on 8 TRN2 NeuronCores.

Algorithm (folded projections; exact math, associativity only):
  S  = (x_q Wq)(x Wk)^T / sqrt(D) = x_q W' x^T,   W' = Wq Wk^T / sqrt(D)
  P  = exp(S) * causal_mask
  O  = (P (x Wv)) / rowsum(P) = ((P x) Wv) / rowsum(P)
K and V are never materialized: the full-T K/V projections (4.3 GF each
per core) are replaced by queries-only projections (2.1 GF each), taking
per-core matmul work from ~16.1 GF to ~9.7 GF.

Sharding: core c = (batch b = c//2, half h = c%2). Queries are split in
32 blocks of 64 rows; slot s (0..15) of a core holds block 31-2s (h=0)
or 30-2s (h=1), so both cores of a batch need the same causal k-tile
prefix CAPS[s] = 16-s per slot (136 [128k x 64q] units total — exactly
the causal minimum for this query interleaving). The causal mask
(qpos >= kpos, on-device) zeroes the two boundary tiles per slot;
interior tiles are provably all-keep. Slot pairs (2p, 2p+1) share a
128-row output projection / rowsum / store.

Per-core stages (all matmul inputs bf16, fp32 PSUM accumulation):
  Q''T[d',q] = sum_d W'[d,d'] xqT[d,q]          (lhsT=W' tile, rhs=xqT)
  ST[k,q]    = sum_d xT[d,k-tile] Q''T[d,q]     (lhsT=xT tile, rhs=Q''T)
  PT[k,q]    = exp(ST) * (qpos[q] >= kpos[k])
  PXT[d,q]   = sum_k x[k,d-chunk] PT[k,q]       (lhsT=x tile, rhs=PT)
  rowsum[q]  = sum_k PT[k,q]                    (lhsT=PT unit, rhs=ones)
  O[q,e]     = sum_d PXT[d,q] Wv[d,e]           (lhsT=PXT tile, rhs=Wv)
  out[q,e]   = O[q,e] / rowsum[q]
"""

import numpy as np
import ml_dtypes

import concourse.bacc as bacc
import concourse.bass as bass
import concourse.mybir as mybir
import concourse.tile as tile
from concourse.bass_utils import run_bass_kernel_spmd

BF16 = mybir.dt.bfloat16
F32 = mybir.dt.float32
F8 = mybir.dt.float8e4
DR = mybir.MatmulPerfMode.DoubleRow
WSCALE = 1024.0  # host rescale of W' so fp8 entries are ~N(0,1)

B, T, D = 4, 2048, 1024
P = 128          # partitions
DT = D // P      # 8 d-tiles
KT_N = T // P    # 16 k-tiles
QB = 64          # query block rows
NSLOT = 16       # slots per core (16 x 64 = 1024 queries)
NQ = NSLOT * QB
CAPS = [16 - i for i in range(NSLOT)]              # k-tile prefix per slot
ASSIGN = {0: [31 - 2 * i for i in range(NSLOT)],   # global q-block per slot
          1: [30 - 2 * i for i in range(NSLOT)]}
SCALE = 1.0 / np.sqrt(np.float32(D))


def _active(kt):
    """number of slots whose cap exceeds kt (slots are cap-descending)"""
    return sum(1 for c in CAPS if c > kt)


# kt-major PT unit layout: unit (kt, s) lives at column UBASE[kt] + s
UBASE = np.cumsum([0] + [_active(k) for k in range(KT_N)]).tolist()
NUNIT = UBASE[KT_N]  # 136

_NC_CACHE = None


def build_nc(repeat=1, hw_loop=True):
    """repeat>1 replays the compute pipeline that many times reusing the
    loaded inputs — used only for differential wall-clock timing."""
    nc = bacc.Bacc("TRN2", target_bir_lowering=False, debug=False,
                   enable_asserts=False, enable_partition_id=False)

    xT8_d = nc.dram_tensor("xT8", [2 * D, T], F8, kind="ExternalInput").ap()
    x8_d = nc.dram_tensor("x8", [2 * T, D], F8, kind="ExternalInput").ap()
    # Q'' operands ship as fp8 hi+lo pairs (stacked on the leading axis):
    # three DoubleRow products (hi.hi + hi.lo + lo.hi) recover bf16-level
    # accuracy at 4x matmul throughput
    xq8_d = nc.dram_tensor("xq8", [2 * D, NQ], F8, kind="ExternalInput").ap()
    Wp8 = nc.dram_tensor("Wp8", [2 * D, D], F8, kind="ExternalInput").ap()
    Wv8 = nc.dram_tensor("Wv8", [2 * D, D], F8, kind="ExternalInput").ap()
    qpos_d = nc.dram_tensor("qpos", [NQ], F32, kind="ExternalInput").ap()
    kpos_d = nc.dram_tensor("kpos", [P, KT_N], F32, kind="ExternalInput").ap()
    out_d = nc.dram_tensor("out", [NQ, D], F32, kind="ExternalOutput").ap()

    with tile.TileContext(nc) as tc:
        with tc.tile_pool(name="sb", bufs=1) as sb, \
             tc.tile_pool(name="ps", bufs=1, space="PSUM") as ps:

            # ---- stage A: load inputs (per-dt tiles so compute streams) ----
            # hi/lo fp8 tiles: index (h*8 + dt) on dim1, h=0 hi, h=1 lo
            _wpr = Wp8.rearrange("(h dt p) e -> p (h dt) e", p=P, h=2)
            _xqr = xq8_d.rearrange("(h dt p) q -> p (h dt) q", p=P, h=2)
            _xtr = xT8_d.rearrange("(h dt p) t -> p (h dt) t", p=P, h=2)
            _xr = x8_d.rearrange("(h kt p) d -> p (h kt) d", p=P, h=2)
            # tiny starter tiles on two sequencers so the first Q'' matmuls
            # (hi.hi of dt-pair 0) are not gated on the full transfers
            wp_start = sb.tile([P, 2, 4 * P], F8, tag="wps", bufs=1, name="wps")
            nc.sync.dma_start(out=wp_start, in_=_wpr[:, 0:2, 0:4 * P])
            xq_start = sb.tile([P, 2, 512], F8, tag="xqs", bufs=1, name="xqs")
            nc.gpsimd.dma_start(out=xq_start, in_=_xqr[:, 0:2, 0:512])
            wp_s = sb.tile([P, 16, D], F8, tag="wp8", bufs=1, name="wp8")
            xq_s = sb.tile([P, 16, NQ], F8, tag="xq8", bufs=1, name="xq8")
            # stream per dt-pair: hi then lo halves of wp and xq
            for j in range(4):
                for h in range(2):
                    nc.sync.dma_start(
                        out=wp_s[:, 8 * h + 2 * j:8 * h + 2 * j + 2, :],
                        in_=_wpr[:, 8 * h + 2 * j:8 * h + 2 * j + 2, :])
                    nc.sync.dma_start(
                        out=xq_s[:, 8 * h + 2 * j:8 * h + 2 * j + 2, :],
                        in_=_xqr[:, 8 * h + 2 * j:8 * h + 2 * j + 2, :])
            kpos_s = sb.tile([P, KT_N], F32, tag="kpos", bufs=1)
            nc.sync.dma_start(out=kpos_s, in_=kpos_d)
            # qpos broadcast is 512KB on the shared DMA bus and is not needed
            # until the first mask (~40us in): issue on the SP queue so it
            # stays ordered behind the critical head DMAs (the Pool sequencer
            # would start it immediately)
            qpos_s = sb.tile([P, NQ], F32, tag="qpos", bufs=1)
            qpos_bcast = bass.AP(tensor=qpos_d.tensor, offset=qpos_d.offset,
                                 ap=[[0, P]] + list(qpos_d.ap))
            nc.sync.dma_start(out=qpos_s, in_=qpos_bcast)
            xT8_s = sb.tile([P, 16, T], F8, tag="xt8", bufs=1, name="xt8")
            for h in range(2):       # all hi chunks first: hh products lead
                for j in range(4):
                    nc.sync.dma_start(
                        out=xT8_s[:, 8 * h + 2 * j:8 * h + 2 * j + 2, :],
                        in_=_xtr[:, 8 * h + 2 * j:8 * h + 2 * j + 2, :])
            x8_s = sb.tile([P, 32, D], F8, tag="x8", bufs=1, name="x8")
            for g in range(4):
                nc.sync.dma_start(out=x8_s[:, 8 * g:8 * (g + 1), :],
                                  in_=_xr[:, 8 * g:8 * (g + 1), :])
            wv_s = sb.tile([P, 16, D], F8, tag="wv8", bufs=1)
            nc.sync.dma_start(
                out=wv_s, in_=Wv8.rearrange("(h dt p) e -> p (h dt) e",
                                            p=P, h=2))
            # ones = 4: folds the px prescale (1/8) and the Wv rescale (x32)
            # back in via the rowsum (out = 4*O / (4*rowsum))
            ones_s = sb.tile([P, 1], BF16, tag="ones", bufs=1)
            nc.vector.memset(ones_s, 4.0)
            # dummy activation: pulls the 1.3us act-table load into the
            # DMA-wait window instead of delaying the first real Act op
            warm_s = sb.tile([P, 1], F32, tag="warm", bufs=1)
            nc.scalar.copy(out=warm_s, in_=ones_s)
            # exp bias -ln8 as a per-partition AP (imm biases need a const AP)
            ebias_s = sb.tile([P, 1], F32, tag="ebias", bufs=1)
            nc.vector.memset(ebias_s, -2.0794415416798357)

            import contextlib
            n_emit = 1 if hw_loop else repeat
            _loop = (tc.For_i(0, repeat, 1) if (hw_loop and repeat > 1)
                     else contextlib.nullcontext())
            with _loop:
              for rep in range(n_emit):
                r = f"_{rep}" if n_emit > 1 else ""
                qt_hi = sb.tile([P, DT, NQ], F8, tag="qth", bufs=1,
                                name=f"qth{r}")
                qt_lo = sb.tile([P, DT, NQ], F8, tag="qtl", bufs=1,
                                name=f"qtl{r}")
                pt_hi = sb.tile([P, NUNIT * QB], F8, tag="pth", bufs=1,
                                name=f"pth{r}")
                pt_lo = sb.tile([P, NUNIT * QB], F8, tag="ptl", bufs=1,
                                name=f"ptl{r}")

                # ---- stage B: Q''T[d',q] via fp8 DoubleRow ----
                # per dt-pair j and et: three 256-contraction products
                # (hi.hi, hi.lo, lo.hi) at 0.5 cycles/row; first four
                # et-chunks stream pair-major behind the DMA arrivals
                def q_mm(pq, et, j, qh, ph, xh, start, stop, use_start):
                    if use_start and ph == 0:
                        lhsT = wp_start[:, :, et * P:(et + 1) * P]
                    else:
                        lhsT = wp_s[:, 8 * ph + 2 * j:8 * ph + 2 * j + 2,
                                    et * P:(et + 1) * P]
                    if use_start and xh == 0 and qh == 0:
                        rhs = xq_start[:, :, 0:512]
                    else:
                        rhs = xq_s[:, 8 * xh + 2 * j:8 * xh + 2 * j + 2,
                                   qh * 512:(qh + 1) * 512]
                    nc.tensor.matmul(pq[:, qh * 512:(qh + 1) * 512],
                                     lhsT, rhs, start=start, stop=stop,
                                     perf_mode=DR)

                pq3 = [ps.tile([P, 1024], F32, tag="big", bufs=4,
                               name=f"pq{r}_{et}") for et in range(4)]
                for j in range(4):
                    us = (j == 0)
                    for et in range(4):
                        for qh in range(2):
                            q_mm(pq3[et], et, j, qh, 0, 0,
                                 start=(j == 0), stop=False, use_start=us)
                    for et in range(4):
                        for qh in range(2):
                            q_mm(pq3[et], et, j, qh, 0, 1,
                                 start=False, stop=False, use_start=us)
                            q_mm(pq3[et], et, j, qh, 1, 0,
                                 start=False, stop=(j == 3), use_start=us)

                def qt_copy(et, pq):
                    # fp8 hi/lo split, pipelined: Act casts half h while DVE
                    # subtracts half h-1, so the psum buf frees fast
                    nc.scalar.copy(out=qt_hi[:, et, 0:512], in_=pq[:, 0:512])
                    nc.vector.tensor_sub(out=qt_lo[:, et, 0:512],
                                         in0=pq[:, 0:512],
                                         in1=qt_hi[:, et, 0:512])
                    nc.scalar.copy(out=qt_hi[:, et, 512:1024],
                                   in_=pq[:, 512:1024])
                    nc.vector.tensor_sub(out=qt_lo[:, et, 512:1024],
                                         in0=pq[:, 512:1024],
                                         in1=qt_hi[:, et, 512:1024])

                for et in range(4):
                    qt_copy(et, pq3[et])
                for et in range(4, DT):
                    pq = ps.tile([P, 1024], F32, tag="big", bufs=4,
                                 name=f"pq{r}_{et}")
                    for j in range(4):
                        for qh in range(2):
                            q_mm(pq, et, j, qh, 0, 0,
                                 start=(j == 0), stop=False, use_start=False)
                            q_mm(pq, et, j, qh, 0, 1,
                                 start=False, stop=False, use_start=False)
                            q_mm(pq, et, j, qh, 1, 0,
                                 start=False, stop=(j == 3), use_start=False)
                    qt_copy(et, pq)

                # ---- stage C: ST = xT.T @ Q''T per k-tile; PT = exp(ST)*mask
                # the two smallest k-tiles are emitted early so their exp
                # latency hides behind the remaining groups instead of
                # stalling the first PX allocations
                _kt_order = list(range(12)) + [14, 15, 12, 13]
                for kt in _kt_order:
                    a = _active(kt)
                    groups = [(0, min(8, a))] + ([(8, a)] if a > 8 else [])
                    for (lo, hi) in groups:
                        w = (hi - lo) * QB
                        st = ps.tile([P, 512], F32, tag="big", bufs=4,
                                     name=f"st{r}_{kt}_{lo}")
                        kc = slice(kt * P, (kt + 1) * P)
                        qc = slice(lo * QB, hi * QB)
                        for j in range(4):
                            nc.tensor.matmul(
                                st[:, 0:w],
                                xT8_s[:, 2 * j:2 * j + 2, kc],
                                qt_hi[:, 2 * j:2 * j + 2, qc],
                                start=(j == 0), stop=False, perf_mode=DR)
                        for j in range(4):
                            nc.tensor.matmul(
                                st[:, 0:w],
                                xT8_s[:, 2 * j:2 * j + 2, kc],
                                qt_lo[:, 2 * j:2 * j + 2, qc],
                                start=False, stop=False, perf_mode=DR)
                            nc.tensor.matmul(
                                st[:, 0:w],
                                xT8_s[:, 8 + 2 * j:8 + 2 * j + 2, kc],
                                qt_hi[:, 2 * j:2 * j + 2, qc],
                                start=False, stop=(j == 3), perf_mode=DR)
                        u0 = UBASE[kt] + lo
                        ptf = sb.tile([P, 512], F32, tag="ptf", bufs=4,
                                      name=f"ptf{r}_{kt}_{lo}")
                        nc.scalar.activation(
                            out=ptf[:, 0:w], in_=st[:, 0:w],
                            func=mybir.ActivationFunctionType.Exp,
                            scale=1.0 / WSCALE)
                        # boundary tiles of the last two active slots need
                        # the causal mask (applied before the hi/lo split);
                        # interior tiles are all-keep
                        for s in (a - 2, a - 1):
                            if not (lo <= s < hi):
                                continue
                            m = sb.tile([P, QB], BF16, tag="mask", bufs=4,
                                        name=f"m{r}_{kt}_{s}")
                            nc.vector.tensor_scalar(
                                out=m, in0=qpos_s[:, s * QB:(s + 1) * QB],
                                scalar1=kpos_s[:, kt:kt + 1], scalar2=None,
                                op0=mybir.AluOpType.is_ge)
                            lc = (s - lo) * QB
                            nc.vector.tensor_mul(
                                out=ptf[:, lc:lc + QB],
                                in0=ptf[:, lc:lc + QB], in1=m)
                        nc.scalar.copy(out=pt_hi[:, u0 * QB:u0 * QB + w],
                                       in_=ptf[:, 0:w])
                        nc.vector.tensor_sub(
                            out=pt_lo[:, u0 * QB:u0 * QB + w],
                            in0=ptf[:, 0:w],
                            in1=pt_hi[:, u0 * QB:u0 * QB + w])

                # ---- stage D/E: PXT accum per slot; rowsum; O = PXT.T @ Wv
                def pt_sl(tile_, s, kt):
                    u = UBASE[kt] + s
                    return tile_[:, u * QB:(u + 1) * QB]

                def pt_pair(tile_, s, m):
                    """[k, 2, QB] view pairing units (2m, s) and (2m+1, s):
                    the kt-major layout gives the pair a constant column
                    stride of _active(2m) units"""
                    base = pt_sl(tile_, s, 2 * m)
                    step = (UBASE[2 * m + 1] - UBASE[2 * m]) * QB
                    return bass.AP(tensor=base.tensor, offset=base.offset,
                                   ap=[list(base.ap[0]), [step, 2],
                                       list(base.ap[1])])

                def emit_px(s, px_sb_pair, psum_pair):
                    """slot s of a pair: even slot -> q cols/rows [0:64] of
                    the pair tiles, odd slot -> [64:128]"""
                    cap = CAPS[s]
                    half = s % 2
                    px = ps.tile([P, DT, QB], F32, tag="big", bufs=4,
                                 name=f"px{r}_{s}")
                    npair, rem = cap // 2, cap % 2
                    # dc outer: a start_tensor_calc marks the whole 2KB psum
                    # zero-region pending, so dc slices sharing a bank must
                    # run their groups sequentially, not interleaved
                    for dc in range(DT):
                        dcc = slice(dc * P, (dc + 1) * P)
                        for m in range(npair):
                            xhi = x8_s[:, 2 * m:2 * m + 2, dcc]
                            xlo = x8_s[:, 16 + 2 * m:16 + 2 * m + 2, dcc]
                            phi = pt_pair(pt_hi, s, m)
                            plo = pt_pair(pt_lo, s, m)
                            lastp = (m == npair - 1 and rem == 0)
                            nc.tensor.matmul(px[:, dc, :], xhi, phi,
                                             start=(m == 0), stop=False,
                                             perf_mode=DR)
                            nc.tensor.matmul(px[:, dc, :], xhi, plo,
                                             start=False, stop=False,
                                             perf_mode=DR)
                            nc.tensor.matmul(px[:, dc, :], xlo, phi,
                                             start=False, stop=lastp,
                                             perf_mode=DR)
                        if rem:
                            kt = cap - 1
                            xhi1 = x8_s[:, kt, dcc]
                            xlo1 = x8_s[:, 16 + kt, dcc]
                            nc.tensor.matmul(px[:, dc, :], xhi1,
                                             pt_sl(pt_hi, s, kt),
                                             start=(npair == 0), stop=False)
                            nc.tensor.matmul(px[:, dc, :], xhi1,
                                             pt_sl(pt_lo, s, kt),
                                             start=False, stop=False)
                            nc.tensor.matmul(px[:, dc, :], xlo1,
                                             pt_sl(pt_hi, s, kt),
                                             start=False, stop=True)
                    # rowsum lands in the pair's partition half (q rows)
                    tgt = psum_pair[half * QB:(half + 1) * QB, :]
                    for i, kt in enumerate(range(cap)):
                        nc.tensor.matmul(tgt, pt_sl(pt_hi, s, kt), ones_s,
                                         start=(i == 0), stop=False)
                    for i, kt in enumerate(range(cap)):
                        nc.tensor.matmul(tgt, pt_sl(pt_lo, s, kt), ones_s,
                                         start=False, stop=(i == cap - 1))
                    # fp8 hi/lo split (prescaled by 1/8 for fp8e4 range);
                    # Act does the hi cast, DVE the residual
                    c0 = half * QB
                    hi_d, lo_d = px_sb_pair
                    nc.scalar.mul(out=hi_d[:, :, c0:c0 + QB], in_=px,
                                  mul=0.125)
                    nc.vector.scalar_tensor_tensor(
                        out=lo_d[:, :, c0:c0 + QB], in0=px, scalar=0.125,
                        in1=hi_d[:, :, c0:c0 + QB],
                        op0=mybir.AluOpType.mult,
                        op1=mybir.AluOpType.subtract)

                def emit_o(p, px_sb, recip, tail=False):
                    o_sb = sb.tile([P, 1024], F32, tag="osb", bufs=2,
                                   name=f"o{r}_{p}")
                    r0 = p * 2 * QB
                    # per-chunk psum tiles: a chunk's group start would
                    # otherwise serialize (tile-level WAR) against the
                    # previous chunk's normalize read; tail=True uses quarter
                    # chunks so the final DMA overlaps remaining matmuls
                    chunks = ([(0, 384), (384, 640), (640, 896), (896, 1024)]
                              if tail else [(0, 512), (512, 1024)])
                    nch = len(chunks)
                    px_hi, px_lo = px_sb
                    for h, (c0, c1) in enumerate(chunks):
                        po = ps.tile([P, c1 - c0], F32, tag="big", bufs=4,
                                     name=f"po{r}_{p}_{h}")
                        ec = slice(c0, c1)
                        for j in range(4):
                            nc.tensor.matmul(
                                po, px_hi[:, 2 * j:2 * j + 2, :],
                                wv_s[:, 2 * j:2 * j + 2, ec],
                                start=(j == 0), stop=False, perf_mode=DR)
                        for j in range(4):
                            nc.tensor.matmul(
                                po, px_hi[:, 2 * j:2 * j + 2, :],
                                wv_s[:, 8 + 2 * j:8 + 2 * j + 2, ec],
                                start=False, stop=False, perf_mode=DR)
                            nc.tensor.matmul(
                                po, px_lo[:, 2 * j:2 * j + 2, :],
                                wv_s[:, 2 * j:2 * j + 2, ec],
                                start=False, stop=(j == 3), perf_mode=DR)
                        # the last chunk takes the faster DVE + SP-queue chain
                        if (nch - 1 - h) % 2 == 0:
                            nc.vector.tensor_scalar_mul(
                                out=o_sb[:, c0:c1], in0=po, scalar1=recip)
                            dma = nc.sync.dma_start
                        else:
                            nc.scalar.mul(out=o_sb[:, c0:c1], in_=po,
                                          mul=recip)
                            dma = nc.scalar.dma_start
                        dma(out=out_d[r0:r0 + 2 * QB, c0:c1],
                            in_=o_sb[:, c0:c1])

                # slots processed small-cap first; a pair's O-projection is
                # emitted one pair later (lag-1) so its px->sbuf copies
                # overlap the next pair's PX matmuls
                pend = None
                cur = None
                for s in range(NSLOT - 1, -1, -1):
                    if s % 2 == 1:
                        p = s // 2
                        px_sb_pair = (
                            sb.tile([P, DT, 2 * QB], F8, tag="pxh", bufs=3,
                                    name=f"pxh{r}_{p}"),
                            sb.tile([P, DT, 2 * QB], F8, tag="pxl", bufs=3,
                                    name=f"pxl{r}_{p}"))
                        psum_pair = ps.tile([P, 1], F32, tag="big", bufs=4,
                                            name=f"psum{r}_{p}")
                        cur = (p, px_sb_pair, psum_pair)
                    emit_px(s, cur[1], cur[2])
                    if s % 2 == 0:
                        p, px_sb_pair, psum_pair = cur
                        recip = sb.tile([P, 1], F32, tag="recip", bufs=4,
                                        name=f"rc{r}_{p}")
                        nc.vector.reciprocal(out=recip, in_=psum_pair)
                        if pend is not None:
                            emit_o(*pend)
                        pend = (p, px_sb_pair, recip)
                emit_o(*pend, tail=True)

    nc.compile()
    return nc


def _split8(a):
    """fp8 hi/lo pair stacked on a new leading axis: a ~= hi + lo"""
    f8 = mybir.dt.np(F8)
    hi = a.astype(f8)
    lo = (a - hi.astype(np.float32)).astype(f8)
    return np.ascontiguousarray(np.stack([hi, lo]))


def _host_prep(x, Wq, Wk, Wv):
    """Build per-core input maps. x: [B,T,D] fp32."""
    bf = ml_dtypes.bfloat16
    Wp8 = _split8((Wq @ Wk.T) * (SCALE * WSCALE))          # [2, D, D]
    Wp8 = Wp8.reshape(2 * D, D)
    Wv8 = _split8(Wv * 32.0).reshape(2 * D, D)   # ~N(0,1): off fp8 subnormals
    kpos = (np.arange(T, dtype=np.float32).reshape(KT_N, P).T).copy()
    xT8_by_batch = [_split8(x[b].T).reshape(2 * D, T) for b in range(B)]
    x8_by_batch = [_split8(x[b]).reshape(2 * T, D) for b in range(B)]
    in_maps = []
    for c in range(8):
        b, h = divmod(c, 2)
        blocks = ASSIGN[h]
        xq = np.concatenate([x[b][j * QB:(j + 1) * QB] for j in blocks],
                            axis=0)
        xq8 = _split8(xq.T).reshape(2 * D, NQ)    # [2*D, 1024] fp8 hi/lo
        qpos = np.concatenate([
            np.arange(j * QB, (j + 1) * QB, dtype=np.float32) for j in blocks])
        in_maps.append({
            "xT8": xT8_by_batch[b], "x8": x8_by_batch[b], "xq8": xq8,
            "Wp8": Wp8, "Wv8": Wv8,
            "qpos": qpos, "kpos": kpos,
        })
    return in_maps


def _reassemble(results, dtype=np.float32):
    out = np.empty((B, T, D), dtype=dtype)
    for c in range(8):
        b, h = divmod(c, 2)
        o = results[c]["out"]                     # [1024, D]
        for s, j in enumerate(ASSIGN[h]):
            out[b, j * QB:(j + 1) * QB] = o[s * QB:(s + 1) * QB]
    return out


def kernel(**inputs):
    global _NC_CACHE
    x = np.asarray(inputs["x"], dtype=np.float32)
    Wq = np.asarray(inputs["Wq"], dtype=np.float32)
    Wk = np.asarray(inputs["Wk"], dtype=np.float32)
    Wv = np.asarray(inputs["Wv"], dtype=np.float32)
    if _NC_CACHE is None:
        _NC_CACHE = build_nc()
    nc = _NC_CACHE
    in_maps = _host_prep(x, Wq, Wk, Wv)
    res = run_bass_kernel_spmd(nc, in_maps, core_ids=list(range(8)))
    return _reassemble(res.results)


if __name__ == "__main__":
    rng = np.random.default_rng(0)
    x = rng.standard_normal((B, T, D), dtype=np.float32)
    Wq = rng.standard_normal((D, D), dtype=np.float32) / np.sqrt(D)
    Wk = rng.standard_normal((D, D), dtype=np.float32) / np.sqrt(D)
    Wv = rng.standard_normal((D, D), dtype=np.float32) / np.sqrt(D)
    out = kernel(x=x, Wq=Wq, Wk=Wk, Wv=Wv)
    print("out", out.shape, out.dtype, np.abs(out).max())
